# revision 1
# baseline (speedup 1.0000x reference)
"""Contrastive projection head loss on 8 Trainium2 NeuronCores.

Reference computation (B=8192, E=1024, P=512):
    z_codon = relu(x[:, :E]) @ w + b          # [B, P]
    z_amino = relu(x[:, E:]) @ w + b          # [B, P]
    z  = concat([z_codon, z_amino], axis=1)   # [B, 2P]
    zn = z / max(||z||, 1e-8)
    s  = (zn @ zn.T);  s[i,i] = -9e15;  s /= 0.1
    nll_i = -s[i, (i - B/2) % B] + logsumexp(s[i, :])
    out = mean(nll)

Distribution: data-parallel over B (1024 rows/core). Each core projects and
normalizes its rows (kept feature-major as zn^T — directly the K-major
operand of the similarity GEMM) and all-gathers zn^T (bf16) in two column
chunks, pipelined with phase 1 by row-halves.

The similarity matrix is symmetric, so each core computes only the block
column range d = 0..4 (its own rows against cores c..c+4 mod 8), halving
the GEMM. Blocks d=1..3 additionally produce column sums of exp(s/T) (one
PE ones-matmul per tile) which are routed to the owning cores with a
ReduceScatter; block d=4 is computed by both endpoints (row sums only), so
every row's logsumexp denominator is covered exactly once. Remote operands
are addressed with partition-id-derived dynamic DMA offsets, keeping the
SPMD program identical on all cores. The self-similarity term is removed
by subtracting its exp (block d=0, compile-time positions); the
positive-pair logit is the block diagonal of the d=4 block.

Returns per-core partial sums [1, 8]; host sums and divides by B.
"""
import numpy as np

from concourse import bass, mybir, tile, bacc
from concourse.bass_utils import run_bass_kernel_spmd
from concourse.masks import make_identity

N_CORES = 8
B = 8192
E = 1024          # embedding size (per half)
P = 512           # projection size
D = 2 * P         # z feature dim = 1024
R = B // N_CORES  # rows per core = 1024
KT = D // 128     # feature sub-tiles = 8
MT = R // 128     # row sub-tiles per core = 8
INV_T = 10.0      # 1 / temperature
EPS = 1e-8

F32 = mybir.dt.float32
F32R = mybir.dt.float32r
BF16 = mybir.dt.bfloat16
GDT = BF16        # dtype of gathered zn^T / similarity-GEMM operands
AF = mybir.ActivationFunctionType
ALU = mybir.AluOpType

NSLOT = 10        # rowsum slots: 2 local (d=0) + 8 remote (d=1..4, k=0..1)

_cached = {}


def _build(no_collective=False):
    nc = bacc.Bacc("TRN2", target_bir_lowering=False, debug=False,
                   enable_asserts=False, num_devices=N_CORES)
    x_in = nc.dram_tensor("xs", [R, 2 * E], F32, kind="ExternalInput").ap()
    w_in = nc.dram_tensor("w", [E, P], F32, kind="ExternalInput").ap()
    b_in = nc.dram_tensor("b", [P], F32, kind="ExternalInput").ap()
    out = nc.dram_tensor("out", [1, MT], F32, kind="ExternalOutput").ap()

    with tile.TileContext(nc) as tc:
        with tc.tile_pool(name="const", bufs=1) as const, \
             tc.tile_pool(name="big", bufs=2) as big, \
             tc.tile_pool(name="small", bufs=1) as small, \
             tc.tile_pool(name="dram", bufs=1, space="DRAM") as dram:

            ident = const.tile([128, 128], F32)
            make_identity(nc, ident[:])
            ones_f = const.tile([128, 1], F32)
            nc.vector.memset(ones_f[:], 1.0)
            ones_r = const.tile([128, 1], F32R)
            nc.vector.tensor_copy(ones_r[:], ones_f[:])
            ones_b = const.tile([128, 1], BF16)
            nc.vector.tensor_copy(ones_b[:], ones_f[:])
            b2 = const.tile([128, P // 128], F32)
            nc.sync.dma_start(b2[:], b_in.rearrange("(mt p) -> p mt", p=128))
            rn_bc = const.tile([128, R], F32)

            # w as [128, KT(=E/128), P] float32r — staged in a scoped pool
            w_r = const.tile([128, E // 128, P], F32R)
            with tc.tile_pool(name="wst", bufs=1) as wst:
                wstage = wst.tile([128, E // 128, P], F32, tag="wstage")
                nc.sync.dma_start(wstage[:],
                                  w_in.rearrange("(kt p) q -> p kt q", p=128))
                nc.vector.tensor_copy(w_r[:], wstage[:])

            # z^T feature-major, f32r; znT is the normalized bf16 copy
            zT = big.tile([128, KT, R], F32R, tag="z")
            znT = big.tile([128, KT, R], GDT, tag="z")
            ag_in = [dram.tile([D, 512], GDT, name=f"ag_in{k}")
                     for k in range(2)]
            ag_out = [dram.tile([N_CORES * D, 512], GDT, name=f"ag_out{k}",
                                addr_space="Local" if no_collective else "Shared")
                      for k in range(2)]
            rn_dram = dram.tile([R], F32)
            rs_in = dram.tile([N_CORES, R], F32)
            rs_out = dram.tile([R], F32)

            # ---- phase 1, pipelined over row-halves jh ----
            with tc.tile_pool(name="xrow", bufs=2) as xrowp, \
                 tc.tile_pool(name="xTp", bufs=2) as xTp, \
                 tc.tile_pool(name="sqp", bufs=2) as sqp, \
                 tc.tile_pool(name="ps1", bufs=2, space="PSUM") as ps1:
                for jh in range(2):
                    # transpose rows of this half (both x halves), with relu
                    xT = xTp.tile([128, 2 * KT, 512], F32R, tag="xT",
                                  name=f"xT{jh}")
                    for r in range(4):
                        rg = jh * 4 + r
                        xrow = xrowp.tile([128, 2 * E], F32, tag="xrow")
                        nc.sync.dma_start(xrow[:],
                                          x_in[rg * 128:(rg + 1) * 128, :])
                        for cg in range(2 * E // 512):
                            pt = ps1.tile([128, 4, 128], F32, tag="tp", bufs=3)
                            for q in range(4):
                                ct = cg * 4 + q
                                nc.tensor.transpose(
                                    pt[:, q, :],
                                    xrow[:, ct * 128:(ct + 1) * 128],
                                    ident[:])
                            nc.vector.tensor_scalar_max(
                                xT[:, cg * 4:(cg + 1) * 4,
                                   r * 128:(r + 1) * 128],
                                pt[:], 0.0)
                    # project this half: zT[:, h*4+m4, jh*512:...]
                    for h in range(2):
                        for m4 in range(P // 128):
                            pz = ps1.tile([128, 512], F32, tag="pz", bufs=2)
                            for kt in range(E // 128):
                                nc.tensor.matmul(
                                    pz[:],
                                    w_r[:, kt, m4 * 128:(m4 + 1) * 128],
                                    xT[:, h * KT + kt, :],
                                    start=(kt == 0), stop=(kt == E // 128 - 1))
                            nc.vector.tensor_scalar(
                                zT[:, h * 4 + m4, jh * 512:(jh + 1) * 512],
                                pz[:], b2[:, m4:m4 + 1], None, ALU.add)
                    # row norms for this half
                    pn = ps1.tile([1, 512], F32, tag="pn", bufs=2,
                                  name=f"pn{jh}")
                    for kt in range(KT):
                        sq = sqp.tile([128, 512], F32R, tag="sq")
                        zsl = zT[:, kt, jh * 512:(jh + 1) * 512]
                        nc.vector.tensor_tensor(sq[:], zsl, zsl, ALU.mult)
                        nc.tensor.matmul(pn[:], ones_r[:], sq[:],
                                         start=(kt == 0), stop=(kt == KT - 1))
                    nrm = small.tile([1, 512], F32, tag="nrm", name=f"nrm{jh}")
                    nc.scalar.activation(nrm[:], pn[:], AF.Sqrt)
                    nc.vector.tensor_scalar_max(nrm[:], nrm[:], EPS)
                    rn_strip = small.tile([1, 512], F32, tag="rns",
                                          name=f"rns{jh}")
                    nc.vector.reciprocal(rn_strip[:], nrm[:])
                    nc.sync.dma_start(rn_dram[None, jh * 512:(jh + 1) * 512],
                                      rn_strip[:])
                    nc.sync.dma_start(
                        rn_bc[:, jh * 512:(jh + 1) * 512],
                        rn_dram[None, jh * 512:(jh + 1) * 512]
                        .to_broadcast([128, 512]))
                    # normalize and ship this half
                    for kt in range(KT):
                        nc.vector.tensor_tensor(
                            znT[:, kt, jh * 512:(jh + 1) * 512],
                            zT[:, kt, jh * 512:(jh + 1) * 512],
                            rn_bc[:, jh * 512:(jh + 1) * 512], ALU.mult)
                    nc.sync.dma_start(
                        ag_in[jh].rearrange("(kt p) j -> p kt j", p=128),
                        znT[:, :, jh * 512:(jh + 1) * 512])
                    if no_collective:
                        for c in range(N_CORES):
                            nc.sync.dma_start(
                                ag_out[jh][c * D:(c + 1) * D, :], ag_in[jh][:])
                    else:
                        nc.gpsimd.collective_compute(
                            "AllGather", ALU.bypass,
                            replica_groups=[list(range(N_CORES))],
                            ins=[ag_in[jh][:]], outs=[ag_out[jh][:]])

            # ---- phase 2: symmetric blockwise cos-sim ----
            rowsum = const.tile([128, MT, NSLOT], F32)
            pos_acc = const.tile([128, MT], F32)
            corr_acc = const.tile([128, MT], F32)

            pid = nc.sync.partition_id()

            def gemm_tile(pg, rhs_ap, m):
                for kt in range(KT):
                    nc.tensor.matmul(pg[:],
                                     znT[:, kt, m * 128:(m + 1) * 128],
                                     rhs_ap[:, kt, :],
                                     start=(kt == 0), stop=(kt == KT - 1))

            with tc.tile_pool(name="rhs", bufs=2) as rhsp, \
                 tc.tile_pool(name="junk", bufs=3) as junkp, \
                 tc.tile_pool(name="dtmp", bufs=4) as dtmpp, \
                 tc.tile_pool(name="ps2", bufs=1, space="PSUM") as ps2:

                # zero the ReduceScatter input (slots we don't write must be 0)
                zb = small.tile([N_CORES, R], F32)
                nc.vector.memset(zb[:], 0.0)
                nc.sync.dma_start(rs_in[:], zb[:])

                # local block d=0 first — overlaps the collectives
                for nb in range(2):
                    for m in range(MT):
                        pg = ps2.tile([128, 512], F32, tag="pg", bufs=5,
                                      name=f"pgl{nb}_{m}")
                        gemm_tile(pg, znT[:, :, nb * 512:(nb + 1) * 512], m)
                        junk = junkp.tile([128, 512], BF16, tag="junk")
                        nc.scalar.activation(
                            junk[:], pg[:], AF.Exp, scale=INV_T,
                            accum_out=rowsum[:, m, nb:nb + 1])
                        if nb == m // 4:
                            # self-similarity at compile-time position
                            off = (m % 4) * 128
                            jd = junkp.tile([128, 128], F32, tag="jd")
                            nc.vector.tensor_tensor(
                                jd[:], pg[:, off:off + 128], ident[:],
                                ALU.mult)
                            d = dtmpp.tile([128, 1], F32, tag="d")
                            nc.vector.reduce_sum(d[:], jd[:],
                                                 axis=mybir.AxisListType.X)
                            nc.scalar.activation(
                                corr_acc[:, m:m + 1], d[:], AF.Exp,
                                scale=INV_T)

                # remote blocks d = 1..4, per gathered chunk k
                for k in range(2):
                    for dd in range(1, 5):
                        slot = 2 + (dd - 1) * 2 + k
                        row0 = ((pid + dd) % N_CORES) * D
                        rhs = rhsp.tile([128, KT, 512], GDT, tag="rhs")
                        src = ag_out[k][bass.ds(row0, D), :].rearrange(
                            "(kt p) j -> p kt j", p=128)
                        nc.sync.dma_start(rhs[:], src)
                        cs = None
                        if dd < 4:
                            cs = ps2.tile([1, 512], F32, tag="cs", bufs=2,
                                          name=f"cs{k}_{dd}")
                        for m in range(MT):
                            pg = ps2.tile([128, 512], F32, tag="pg", bufs=5,
                                          name=f"pg{k}_{dd}_{m}")
                            gemm_tile(pg, rhs, m)
                            junk = junkp.tile([128, 512], BF16, tag="junk")
                            nc.scalar.activation(
                                junk[:], pg[:], AF.Exp, scale=INV_T,
                                accum_out=rowsum[:, m, slot:slot + 1])
                            if dd < 4:
                                nc.tensor.matmul(cs[:], ones_b[:], junk[:],
                                                 start=(m == 0),
                                                 stop=(m == MT - 1))
                            if dd == 4 and k == m // 4:
                                # positive-pair logit on the block diagonal
                                off = (m % 4) * 128
                                jd = junkp.tile([128, 128], F32, tag="jd")
                                nc.vector.tensor_tensor(
                                    jd[:], pg[:, off:off + 128], ident[:],
                                    ALU.mult)
                                dpos = dtmpp.tile([128, 1], F32, tag="dp")
                                nc.vector.reduce_sum(
                                    dpos[:], jd[:], axis=mybir.AxisListType.X)
                                nc.vector.tensor_scalar_mul(
                                    pos_acc[:, m:m + 1], dpos[:], INV_T)
                        if dd < 4:
                            # ship this block's column sums to core (c+dd)
                            css = dtmpp.tile([1, 512], F32, tag="css",
                                             name=f"css{k}_{dd}")
                            nc.vector.tensor_copy(css[:], cs[:])
                            nc.sync.dma_start(
                                rs_in[bass.ds((pid + dd) % N_CORES, 1),
                                      k * 512:(k + 1) * 512],
                                css[:])

                # sum exchanged column contributions
                if no_collective:
                    nc.sync.dma_start(rs_out[None, :], rs_in[0:1, :])
                else:
                    nc.gpsimd.collective_compute(
                        "ReduceScatter", ALU.add,
                        replica_groups=[list(range(N_CORES))],
                        ins=[rs_in[:]], outs=[rs_out[:]])

                # ---- finale: lse, nll, partial sum (batched over m) ----
                rs = small.tile([128, MT], F32)
                nc.vector.reduce_sum(rs[:], rowsum[:],
                                     axis=mybir.AxisListType.X)
                rcv = small.tile([128, MT], F32)
                nc.sync.dma_start(rcv[:], rs_out.rearrange("(m p) -> p m", p=128))
                nc.vector.tensor_tensor(rs[:], rs[:], rcv[:], ALU.add)
                nc.vector.tensor_tensor(rs[:], rs[:], corr_acc[:], ALU.subtract)
                lse = small.tile([128, MT], F32)
                nc.scalar.activation(lse[:], rs[:], AF.Ln)
                nll = small.tile([128, MT], F32)
                nc.vector.tensor_tensor(nll[:], lse[:], pos_acc[:], ALU.subtract)
                pf = ps2.tile([1, MT], F32, tag="pf", bufs=1)
                nc.tensor.matmul(pf[:], ones_f[:], nll[:], start=True, stop=True)
                fs = small.tile([1, MT], F32)
                nc.vector.tensor_copy(fs[:], pf[:])
                nc.sync.dma_start(out[:], fs[:])

    nc.compile()
    return nc


def kernel(x, w, b):
    if "nc" not in _cached:
        _cached["nc"] = _build()
    nc = _cached["nc"]
    x = np.ascontiguousarray(np.asarray(x, dtype=np.float32))
    w = np.ascontiguousarray(np.asarray(w, dtype=np.float32))
    b = np.ascontiguousarray(np.asarray(b, dtype=np.float32))
    in_maps = [{
        "xs": np.ascontiguousarray(x[c * R:(c + 1) * R]),
        "w": w, "b": b,
    } for c in range(N_CORES)]
    res = run_bass_kernel_spmd(nc, in_maps, list(range(N_CORES)))
    total = 0.0
    for c in range(N_CORES):
        total += float(res.results[c]["out"].astype(np.float64).sum())
    return np.float32(total / B)



# revision 5
# speedup vs baseline: 1.1218x; 1.1218x over previous
"""Contrastive projection head loss on 8 Trainium2 NeuronCores.

Reference computation (B=8192, E=1024, P=512):
    z_codon = relu(x[:, :E]) @ w + b          # [B, P]
    z_amino = relu(x[:, E:]) @ w + b          # [B, P]
    z  = concat([z_codon, z_amino], axis=1)   # [B, 2P]
    zn = z / max(||z||, 1e-8)
    s  = (zn @ zn.T);  s[i,i] = -9e15;  s /= 0.1
    nll_i = -s[i, (i - B/2) % B] + logsumexp(s[i, :])
    out = mean(nll)

Distribution: data-parallel over B (1024 rows/core). Each core projects and
normalizes its rows, keeping them feature-major (zn^T) as the K-major
operand of the similarity GEMM, and all-gathers zn^T in fp8 (scaled by 16)
in two column chunks pipelined with phase 1 by row-halves.

Phase 1 runs in bf16: x rows are relu'd+cast on DVE, transposed by the DMA
XBAR (16x128 tiles, no PE involvement), projected with a bf16 GEMM, and
normalized via a PE ones-broadcast of 16/||z|| (no DRAM round-trip).

The similarity matrix is symmetric, so each core computes only block
columns d = 0..4 (its own rows against cores c..c+4 mod 8). The GEMM runs
in fp8e4 with MatmulPerfMode.DoubleRow (two 128-K tiles per instruction).
Since operands carry a 16x scale, exp scales use 1/(T*16^2). Blocks d=1..3
also produce column sums of exp (ones-matmul per tile) routed to the
owning cores with a ReduceScatter; block d=4 is computed by both endpoints
(row sums only). Remote operands are addressed with partition-id-derived
dynamic DMA offsets, keeping the SPMD program identical on all cores. The
self-similarity term is removed by subtracting its exp (block d=0,
compile-time positions); the positive-pair logit is the block diagonal of
the d=4 block.

Returns per-core partial sums [1, 8]; host sums and divides by B.
"""
import numpy as np

from concourse import bass, mybir, tile, bacc
from concourse.bass_utils import run_bass_kernel_spmd
from concourse.masks import make_identity

N_CORES = 8
B = 8192
E = 1024          # embedding size (per half)
P = 512           # projection size
D = 2 * P         # z feature dim = 1024
R = B // N_CORES  # rows per core = 1024
KT = D // 128     # feature sub-tiles = 8
MT = R // 128     # row sub-tiles per core = 8
INV_T = 10.0      # 1 / temperature
S = 16.0          # fp8 operand scale: znT holds 16*zn
SCL = INV_T / (S * S)   # exp input scale: logits = SCL * (16zn . 16zn)
EPS = 1e-8

F32 = mybir.dt.float32
F32R = mybir.dt.float32r
BF16 = mybir.dt.bfloat16
GDT = mybir.dt.float8e4   # gathered zn^T / similarity-GEMM operand dtype
AF = mybir.ActivationFunctionType
ALU = mybir.AluOpType
DR = mybir.MatmulPerfMode.DoubleRow

NSLOT = 10        # rowsum slots: 2 local (d=0) + 8 remote (d=1..4, k=0..1)

_cached = {}


def _build(no_collective=False):
    nc = bacc.Bacc("TRN2", target_bir_lowering=False, debug=False,
                   enable_asserts=False, num_devices=N_CORES)
    x_in = nc.dram_tensor("xs", [R, 2 * E], F32, kind="ExternalInput").ap()
    w_in = nc.dram_tensor("w", [E, P], F32, kind="ExternalInput").ap()
    b_in = nc.dram_tensor("b", [P], F32, kind="ExternalInput").ap()
    out = nc.dram_tensor("out", [1, MT], F32, kind="ExternalOutput").ap()

    with tile.TileContext(nc) as tc:
        with tc.tile_pool(name="const", bufs=1) as const, \
             tc.tile_pool(name="big", bufs=1) as big, \
             tc.tile_pool(name="small", bufs=1) as small, \
             tc.tile_pool(name="dram", bufs=1, space="DRAM") as dram:

            ident = const.tile([128, 128], F32)
            make_identity(nc, ident[:])
            ones_f = const.tile([128, 1], F32)
            nc.vector.memset(ones_f[:], 1.0)
            ones_b = const.tile([128, 1], BF16)
            nc.vector.tensor_copy(ones_b[:], ones_f[:])
            # row-vector of ones for the PE norm broadcast (K=1 matmul)
            onesr_f = const.tile([1, 128], F32)
            nc.vector.memset(onesr_f[:], 1.0)
            onesr = const.tile([1, 128], F32R)
            nc.vector.tensor_copy(onesr[:], onesr_f[:])
            b2 = const.tile([128, P // 128], F32)
            nc.sync.dma_start(b2[:], b_in.rearrange("(mt p) -> p mt", p=128))

            # w as [128, KT(=E/128), P] bf16 — staged in a scoped pool
            w_b = const.tile([128, E // 128, P], BF16)
            with tc.tile_pool(name="wst", bufs=1) as wst:
                wstage = wst.tile([128, E // 128, P], F32, tag="wstage")
                nc.scalar.dma_start(wstage[:],
                                    w_in.rearrange("(kt p) q -> p kt q", p=128))
                nc.vector.tensor_copy(w_b[:], wstage[:])

            # z^T feature-major bf16; znT is the normalized 16x-scaled fp8
            zT = big.tile([128, KT, R], BF16, tag="zT")
            znT = big.tile([128, KT, R], GDT, tag="znT")
            ag_in = [dram.tile([D, 512], GDT, name=f"ag_in{k}")
                     for k in range(2)]
            ag_out = [dram.tile([N_CORES * D, 512], GDT, name=f"ag_out{k}",
                                addr_space="Local" if no_collective else "Shared")
                      for k in range(2)]
            rs_in = dram.tile([N_CORES, R], F32)
            rs_out = dram.tile([R], F32)

            # ---- phase 1, pipelined over row-halves jh ----
            with tc.tile_pool(name="xrow", bufs=2) as xrowp, \
                 tc.tile_pool(name="xbf", bufs=2) as xbfp, \
                 tc.tile_pool(name="xTp", bufs=2) as xTp, \
                 tc.tile_pool(name="sqp", bufs=2) as sqp, \
                 tc.tile_pool(name="ps1", bufs=2, space="PSUM") as ps1:
                for jh in range(2):
                    # load + relu-cast + XBAR-transpose this half's rows
                    xT = xTp.tile([128, 2 * KT, 512], BF16, tag="xT",
                                  name=f"xT{jh}")
                    for r in range(4):
                        rg = jh * 4 + r
                        ld = nc.sync if rg % 2 == 0 else nc.scalar
                        tr = nc.scalar if rg % 2 == 0 else nc.sync
                        xrow = xrowp.tile([128, 2 * E], F32, tag="xrow")
                        ld.dma_start(xrow[:], x_in[rg * 128:(rg + 1) * 128, :])
                        xbf = xbfp.tile([128, 2 * E], BF16, tag="xbf")
                        nc.vector.tensor_scalar_max(xbf[:], xrow[:], 0.0)
                        tr.dma_start(xT[:, :, r * 128:(r + 1) * 128], xbf[:],
                                     transpose=True)
                    # project this half: zT[:, h*4+m4, jh*512:...]
                    for h in range(2):
                        for m4 in range(P // 128):
                            pz = ps1.tile([128, 512], F32, tag="pz", bufs=2)
                            for kt in range(E // 128):
                                nc.tensor.matmul(
                                    pz[:],
                                    w_b[:, kt, m4 * 128:(m4 + 1) * 128],
                                    xT[:, h * KT + kt, :],
                                    start=(kt == 0), stop=(kt == E // 128 - 1))
                            nc.vector.tensor_scalar(
                                zT[:, h * 4 + m4, jh * 512:(jh + 1) * 512],
                                pz[:], b2[:, m4:m4 + 1], None, ALU.add)
                    # row norms for this half (bf16 squares, PE reduce)
                    pn = ps1.tile([1, 512], F32, tag="pn", bufs=2,
                                  name=f"pn{jh}")
                    for kt in range(KT):
                        sq = sqp.tile([128, 512], BF16, tag="sq")
                        zsl = zT[:, kt, jh * 512:(jh + 1) * 512]
                        nc.vector.tensor_tensor(sq[:], zsl, zsl, ALU.mult)
                        nc.tensor.matmul(pn[:], ones_b[:], sq[:],
                                         start=(kt == 0), stop=(kt == KT - 1))
                    # nrm = ||z|| / S  (Sqrt with input scale 1/S^2)
                    nrm = small.tile([1, 512], F32, tag="nrm", name=f"nrm{jh}")
                    nc.scalar.activation(nrm[:], pn[:], AF.Sqrt,
                                         scale=1.0 / (S * S))
                    nc.vector.tensor_scalar_max(nrm[:], nrm[:], EPS)
                    rn = small.tile([1, 512], F32R, tag="rns", name=f"rns{jh}")
                    with nc.allow_low_precision(reason="f32r is f32 bitwise"):
                        nc.vector.reciprocal(rn[:], nrm[:])   # = S / ||z||
                    # broadcast across partitions via K=1 PE matmul
                    rnb = ps1.tile([128, 512], F32, tag="rnb", bufs=2,
                                   name=f"rnb{jh}")
                    nc.tensor.matmul(rnb[:], onesr[:], rn[:],
                                     start=True, stop=True)
                    # normalize (x16) to fp8 and ship this half
                    for kt in range(KT):
                        nc.vector.tensor_tensor(
                            znT[:, kt, jh * 512:(jh + 1) * 512],
                            zT[:, kt, jh * 512:(jh + 1) * 512],
                            rnb[:], ALU.mult)
                    nc.sync.dma_start(
                        ag_in[jh].rearrange("(kt p) j -> p kt j", p=128),
                        znT[:, :, jh * 512:(jh + 1) * 512])
                    if no_collective:
                        for c in range(N_CORES):
                            nc.sync.dma_start(
                                ag_out[jh][c * D:(c + 1) * D, :], ag_in[jh][:])
                    else:
                        nc.gpsimd.collective_compute(
                            "AllGather", ALU.bypass,
                            replica_groups=[list(range(N_CORES))],
                            ins=[ag_in[jh][:]], outs=[ag_out[jh][:]])

            # ---- phase 2: symmetric blockwise cos-sim (fp8 DoubleRow) ----
            rowsum = const.tile([128, MT, NSLOT], F32)
            pos_acc = const.tile([128, MT], F32)
            corr_acc = const.tile([128, MT], F32)

            pid = nc.sync.partition_id()
            pid_act = nc.scalar.partition_id()

            def gemm_tile(pg, rhs_ap, m):
                for t in range(KT // 2):
                    nc.tensor.matmul(
                        pg[:],
                        znT[:, 2 * t:2 * t + 2, m * 128:(m + 1) * 128],
                        rhs_ap[:, 2 * t:2 * t + 2, :],
                        start=(t == 0), stop=(t == KT // 2 - 1),
                        perf_mode=DR)

            with tc.tile_pool(name="rhs", bufs=2) as rhsp, \
                 tc.tile_pool(name="junk", bufs=3) as junkp, \
                 tc.tile_pool(name="dtmp", bufs=4) as dtmpp, \
                 tc.tile_pool(name="ps2", bufs=1, space="PSUM") as ps2:

                # zero the ReduceScatter input (slots we don't write must be 0)
                zb = small.tile([N_CORES, R], F32)
                nc.vector.memset(zb[:], 0.0)
                nc.sync.dma_start(rs_in[:], zb[:])

                # local block d=0 first — overlaps the collectives
                for nb in range(2):
                    for m in range(MT):
                        pg = ps2.tile([128, 512], F32, tag="pg", bufs=5,
                                      name=f"pgl{nb}_{m}")
                        gemm_tile(pg, znT[:, :, nb * 512:(nb + 1) * 512], m)
                        junk = junkp.tile([128, 512], BF16, tag="junk")
                        nc.scalar.activation(
                            junk[:], pg[:], AF.Exp, scale=SCL,
                            accum_out=rowsum[:, m, nb:nb + 1])
                        if nb == m // 4:
                            # self-similarity at compile-time position
                            off = (m % 4) * 128
                            jd = junkp.tile([128, 128], F32, tag="jd")
                            nc.vector.tensor_tensor(
                                jd[:], pg[:, off:off + 128], ident[:],
                                ALU.mult)
                            d = dtmpp.tile([128, 1], F32, tag="d")
                            nc.vector.reduce_sum(d[:], jd[:],
                                                 axis=mybir.AxisListType.X)
                            nc.scalar.activation(
                                corr_acc[:, m:m + 1], d[:], AF.Exp,
                                scale=SCL)

                # remote blocks d = 1..4, per gathered chunk k
                for k in range(2):
                    for dd in range(1, 5):
                        slot = 2 + (dd - 1) * 2 + k
                        eng = nc.sync if dd % 2 == 0 else nc.scalar
                        epid = pid if dd % 2 == 0 else pid_act
                        row0 = ((epid + dd) % N_CORES) * D
                        rhs = rhsp.tile([128, KT, 512], GDT, tag="rhs")
                        src = ag_out[k][bass.ds(row0, D), :].rearrange(
                            "(kt p) j -> p kt j", p=128)
                        eng.dma_start(rhs[:], src)
                        cs = None
                        if dd < 4:
                            cs = ps2.tile([1, 512], F32, tag="cs", bufs=2,
                                          name=f"cs{k}_{dd}")
                        for m in range(MT):
                            pg = ps2.tile([128, 512], F32, tag="pg", bufs=5,
                                          name=f"pg{k}_{dd}_{m}")
                            gemm_tile(pg, rhs, m)
                            junk = junkp.tile([128, 512], BF16, tag="junk")
                            nc.scalar.activation(
                                junk[:], pg[:], AF.Exp, scale=SCL,
                                accum_out=rowsum[:, m, slot:slot + 1])
                            if dd < 4:
                                nc.tensor.matmul(cs[:], ones_b[:], junk[:],
                                                 start=(m == 0),
                                                 stop=(m == MT - 1))
                            if dd == 4 and k == m // 4:
                                # positive-pair logit on the block diagonal
                                off = (m % 4) * 128
                                jd = junkp.tile([128, 128], F32, tag="jd")
                                nc.vector.tensor_tensor(
                                    jd[:], pg[:, off:off + 128], ident[:],
                                    ALU.mult)
                                dpos = dtmpp.tile([128, 1], F32, tag="dp")
                                nc.vector.reduce_sum(
                                    dpos[:], jd[:], axis=mybir.AxisListType.X)
                                nc.vector.tensor_scalar_mul(
                                    pos_acc[:, m:m + 1], dpos[:], SCL)
                        if dd < 4:
                            # ship this block's column sums to core (c+dd)
                            css = dtmpp.tile([1, 512], F32, tag="css",
                                             name=f"css{k}_{dd}")
                            nc.vector.tensor_copy(css[:], cs[:])
                            nc.sync.dma_start(
                                rs_in[bass.ds((pid + dd) % N_CORES, 1),
                                      k * 512:(k + 1) * 512],
                                css[:])

                # sum exchanged column contributions
                if no_collective:
                    nc.sync.dma_start(rs_out[None, :], rs_in[0:1, :])
                else:
                    nc.gpsimd.collective_compute(
                        "ReduceScatter", ALU.add,
                        replica_groups=[list(range(N_CORES))],
                        ins=[rs_in[:]], outs=[rs_out[:]])

                # ---- finale: lse, nll, partial sum (batched over m) ----
                rs = small.tile([128, MT], F32)
                nc.vector.reduce_sum(rs[:], rowsum[:],
                                     axis=mybir.AxisListType.X)
                rcv = small.tile([128, MT], F32)
                nc.sync.dma_start(rcv[:], rs_out.rearrange("(m p) -> p m", p=128))
                nc.vector.tensor_tensor(rs[:], rs[:], rcv[:], ALU.add)
                nc.vector.tensor_tensor(rs[:], rs[:], corr_acc[:], ALU.subtract)
                lse = small.tile([128, MT], F32)
                nc.scalar.activation(lse[:], rs[:], AF.Ln)
                nll = small.tile([128, MT], F32)
                nc.vector.tensor_tensor(nll[:], lse[:], pos_acc[:], ALU.subtract)
                pf = ps2.tile([1, MT], F32, tag="pf", bufs=1)
                nc.tensor.matmul(pf[:], ones_f[:], nll[:], start=True, stop=True)
                fs = small.tile([1, MT], F32)
                nc.vector.tensor_copy(fs[:], pf[:])
                nc.sync.dma_start(out[:], fs[:])

    nc.compile()
    return nc


def kernel(x, w, b):
    if "nc" not in _cached:
        _cached["nc"] = _build()
    nc = _cached["nc"]
    x = np.ascontiguousarray(np.asarray(x, dtype=np.float32))
    w = np.ascontiguousarray(np.asarray(w, dtype=np.float32))
    b = np.ascontiguousarray(np.asarray(b, dtype=np.float32))
    in_maps = [{
        "xs": np.ascontiguousarray(x[c * R:(c + 1) * R]),
        "w": w, "b": b,
    } for c in range(N_CORES)]
    res = run_bass_kernel_spmd(nc, in_maps, list(range(N_CORES)))
    total = 0.0
    for c in range(N_CORES):
        total += float(res.results[c]["out"].astype(np.float64).sum())
    return np.float32(total / B)


# revision 6
# speedup vs baseline: 1.2931x; 1.1527x over previous
"""Contrastive projection head loss on 8 Trainium2 NeuronCores.

Reference computation (B=8192, E=1024, P=512):
    z_codon = relu(x[:, :E]) @ w + b          # [B, P]
    z_amino = relu(x[:, E:]) @ w + b          # [B, P]
    z  = concat([z_codon, z_amino], axis=1)   # [B, 2P]
    zn = z / max(||z||, 1e-8)
    s  = (zn @ zn.T);  s[i,i] = -9e15;  s /= 0.1
    nll_i = -s[i, (i - B/2) % B] + logsumexp(s[i, :])
    out = mean(nll)

Distribution: data-parallel over B (1024 rows/core). Each core projects and
normalizes its rows, keeping them feature-major (zn^T) as the K-major
operand of the similarity GEMM, and all-gathers zn^T in fp8 (scaled by 16)
in two column chunks pipelined with phase 1 by row-halves.

Phase 1 runs in bf16: x rows are relu'd+cast on DVE, transposed on the PE
(bf16 identity, 1 cycle/row), projected with a bf16 GEMM. 16/||z|| is
computed with a Quake-style rsqrt (bit trick + 2 Newton steps) entirely on
DVE so the Scalar engine's activation table stays on Exp the whole kernel
(table swaps cost ~2-11us), then broadcast across partitions with a K=1 PE
matmul (no DRAM round-trip).

The similarity matrix is symmetric, so each core computes only block
columns d = 0..4 (its own rows against cores c..c+4 mod 8). The GEMM runs
in fp8e4 with MatmulPerfMode.DoubleRow (two 128-K tiles per instruction).
Since operands carry a 16x scale, exp scales use 1/(T*16^2). Blocks d=1..3
also produce column sums of exp (ones-matmul per tile) routed to the
owning cores with a ReduceScatter issued BEFORE the final d=4 block so it
overlaps compute; block d=4 is computed by both endpoints (row sums only).
Remote operands are addressed with partition-id-derived dynamic DMA
offsets, keeping the SPMD program identical on all cores. The
self-similarity term is removed by subtracting its exp (block d=0,
compile-time positions); the positive-pair logit is the block diagonal of
the d=4 block. A tiny warmup AllGather at program start absorbs one-time
collective setup latency, and a dummy Exp preloads the activation table.

Returns per-core partial sums [1, 8]; host sums and divides by B.
"""
import numpy as np

from concourse import bass, mybir, tile, bacc
from concourse.bass_utils import run_bass_kernel_spmd
from concourse.masks import make_identity

N_CORES = 8
B = 8192
E = 1024          # embedding size (per half)
P = 512           # projection size
D = 2 * P         # z feature dim = 1024
R = B // N_CORES  # rows per core = 1024
KT = D // 128     # feature sub-tiles = 8
MT = R // 128     # row sub-tiles per core = 8
INV_T = 10.0      # 1 / temperature
S = 16.0          # fp8 operand scale: znT holds 16*zn
SCL = INV_T / (S * S)   # exp input scale: logits = SCL * (16zn . 16zn)
MAGIC = 0x5F3759DF      # Quake rsqrt seed constant

F32 = mybir.dt.float32
F32R = mybir.dt.float32r
I32 = mybir.dt.int32
BF16 = mybir.dt.bfloat16
GDT = mybir.dt.float8e4   # gathered zn^T / similarity-GEMM operand dtype
AF = mybir.ActivationFunctionType
ALU = mybir.AluOpType
DR = mybir.MatmulPerfMode.DoubleRow

NSLOT = 10        # rowsum slots: 2 local (d=0) + 8 remote (d=1..4, k=0..1)

_cached = {}


def _build(no_collective=False):
    nc = bacc.Bacc("TRN2", target_bir_lowering=False, debug=False,
                   enable_asserts=False, num_devices=N_CORES)
    x_in = nc.dram_tensor("xs", [R, 2 * E], F32, kind="ExternalInput").ap()
    w_in = nc.dram_tensor("w", [E, P], F32, kind="ExternalInput").ap()
    b_in = nc.dram_tensor("b", [P], F32, kind="ExternalInput").ap()
    out = nc.dram_tensor("out", [1, MT], F32, kind="ExternalOutput").ap()

    with tile.TileContext(nc) as tc:
        with tc.tile_pool(name="const", bufs=1) as const, \
             tc.tile_pool(name="big", bufs=1) as big, \
             tc.tile_pool(name="small", bufs=1) as small, \
             tc.tile_pool(name="dram", bufs=1, space="DRAM") as dram:

            # warmup: preload the Exp activation table while DMAs run
            dumm = const.tile([1, 8], F32)
            nc.vector.memset(dumm[:], 0.0)
            dumm2 = const.tile([1, 8], F32)
            nc.scalar.activation(dumm2[:], dumm[:], AF.Exp)

            ident = const.tile([128, 128], F32)
            make_identity(nc, ident[:])
            ident_b = const.tile([128, 128], BF16)
            nc.vector.tensor_copy(ident_b[:], ident[:])
            ones_f = const.tile([128, 1], F32)
            nc.vector.memset(ones_f[:], 1.0)
            ones_b = const.tile([128, 1], BF16)
            nc.vector.tensor_copy(ones_b[:], ones_f[:])
            # row-vector of ones for the PE norm broadcast (K=1 matmul)
            onesr_f = const.tile([1, 128], F32)
            nc.vector.memset(onesr_f[:], 1.0)
            onesr = const.tile([1, 128], F32R)
            nc.vector.tensor_copy(onesr[:], onesr_f[:])
            magic = const.tile([1, 512], I32)
            nc.vector.memset(magic[:], MAGIC)
            b2 = const.tile([128, P // 128], F32)
            nc.sync.dma_start(b2[:], b_in.rearrange("(mt p) -> p mt", p=128))

            # w as [128, KT(=E/128), P] bf16 — staged in a scoped pool
            w_b = const.tile([128, E // 128, P], BF16)
            with tc.tile_pool(name="wst", bufs=1) as wst:
                wstage = wst.tile([128, E // 128, P], F32, tag="wstage")
                nc.scalar.dma_start(wstage[:],
                                    w_in.rearrange("(kt p) q -> p kt q", p=128))
                nc.vector.tensor_copy(w_b[:], wstage[:])

            # z^T feature-major bf16; znT is the normalized 16x-scaled fp8
            zT = big.tile([128, KT, R], BF16, tag="zT")
            znT = big.tile([128, KT, R], GDT, tag="znT")
            ag_in = [dram.tile([D, 512], GDT, name=f"ag_in{k}")
                     for k in range(2)]
            ag_out = [dram.tile([N_CORES * D, 512], GDT, name=f"ag_out{k}",
                                addr_space="Local" if no_collective else "Shared")
                      for k in range(2)]
            warm_in = dram.tile([8, 4], F32)
            warm_out = dram.tile([N_CORES * 8, 4], F32,
                                 addr_space="Local" if no_collective else "Shared")
            rs_in = dram.tile([N_CORES, R], F32)
            rs_out = dram.tile([R], F32)

            # warmup collective: absorb one-time CC setup latency
            wz = small.tile([8, 4], F32)
            nc.vector.memset(wz[:], 0.0)
            nc.sync.dma_start(warm_in[:], wz[:])
            if not no_collective:
                nc.gpsimd.collective_compute(
                    "AllGather", ALU.bypass,
                    replica_groups=[list(range(N_CORES))],
                    ins=[warm_in[:]], outs=[warm_out[:]])

            # zero the ReduceScatter input (slots we don't write must be 0)
            zb = small.tile([N_CORES, R], F32)
            nc.vector.memset(zb[:], 0.0)
            nc.sync.dma_start(rs_in[:], zb[:])

            # ---- phase 1, pipelined over row-halves jh ----
            with tc.tile_pool(name="xrow", bufs=3) as xrowp, \
                 tc.tile_pool(name="xbf", bufs=3) as xbfp, \
                 tc.tile_pool(name="xTp", bufs=2) as xTp, \
                 tc.tile_pool(name="sqp", bufs=2) as sqp, \
                 tc.tile_pool(name="ps1", bufs=1, space="PSUM") as ps1:
                for jh in range(2):
                    # load + relu-cast + PE-transpose this half's rows
                    xT = xTp.tile([128, 2 * KT, 512], BF16, tag="xT",
                                  name=f"xT{jh}")
                    for r in range(4):
                        rg = jh * 4 + r
                        ld = nc.sync if rg % 2 == 0 else nc.scalar
                        xrow = xrowp.tile([128, 2 * E], F32, tag="xrow")
                        ld.dma_start(xrow[:], x_in[rg * 128:(rg + 1) * 128, :])
                        xbf = xbfp.tile([128, 2 * E], BF16, tag="xbf")
                        nc.vector.tensor_scalar_max(xbf[:], xrow[:], 0.0)
                        for cg in range(2 * E // 512):
                            pt = ps1.tile([128, 4, 128], BF16, tag="tp",
                                          bufs=3)
                            for q in range(4):
                                ct = cg * 4 + q
                                nc.tensor.transpose(
                                    pt[:, q, :],
                                    xbf[:, ct * 128:(ct + 1) * 128],
                                    ident_b[:])
                            nc.vector.tensor_copy(
                                xT[:, cg * 4:(cg + 1) * 4,
                                   r * 128:(r + 1) * 128],
                                pt[:])
                    # project this half: zT[:, h*4+m4, jh*512:...]
                    for h in range(2):
                        for m4 in range(P // 128):
                            pz = ps1.tile([128, 512], F32, tag="pz", bufs=2)
                            for kt in range(E // 128):
                                nc.tensor.matmul(
                                    pz[:],
                                    w_b[:, kt, m4 * 128:(m4 + 1) * 128],
                                    xT[:, h * KT + kt, :],
                                    start=(kt == 0), stop=(kt == E // 128 - 1))
                            nc.vector.tensor_scalar(
                                zT[:, h * 4 + m4, jh * 512:(jh + 1) * 512],
                                pz[:], b2[:, m4:m4 + 1], None, ALU.add)
                    # row norms for this half (bf16 squares, PE reduce)
                    pn = ps1.tile([1, 512], F32, tag="pn", bufs=2,
                                  name=f"pn{jh}")
                    for kt in range(KT):
                        sq = sqp.tile([128, 512], BF16, tag="sq")
                        zsl = zT[:, kt, jh * 512:(jh + 1) * 512]
                        nc.vector.tensor_tensor(sq[:], zsl, zsl, ALU.mult)
                        nc.tensor.matmul(pn[:], ones_b[:], sq[:],
                                         start=(kt == 0), stop=(kt == KT - 1))
                    # rn = S/||z|| = rsqrt(pn/S^2): Quake seed + 2 Newton
                    # steps, all on DVE (keeps ACT's table on Exp)
                    scr = small.tile([1, 8, 512], F32, tag="scr",
                                     name=f"scr{jh}")
                    pnf, hh, t1, u1, w1, y1, y2 = (scr[:, i, :]
                                                   for i in range(7))
                    nc.vector.tensor_scalar_mul(pnf, pn[:], 1.0 / (S * S))
                    nc.vector.tensor_scalar_mul(hh, pn[:], 0.5 / (S * S))
                    nc.vector.tensor_scalar(u1.bitcast(I32), pnf.bitcast(I32),
                                            1, None, ALU.arith_shift_right)
                    nc.vector.tensor_tensor(t1.bitcast(I32), magic[:],
                                            u1.bitcast(I32), ALU.subtract)
                    # t1 now holds y0; iterate y <- y*(1.5 - h*y*y)
                    nc.vector.tensor_tensor(u1, t1, t1, ALU.mult)
                    nc.vector.tensor_tensor(w1, u1, hh, ALU.mult)
                    nc.vector.tensor_scalar(w1, w1, -1.0, 1.5, ALU.mult,
                                            ALU.add)
                    nc.vector.tensor_tensor(y1, t1, w1, ALU.mult)
                    nc.vector.tensor_tensor(u1, y1, y1, ALU.mult)
                    nc.vector.tensor_tensor(w1, u1, hh, ALU.mult)
                    nc.vector.tensor_scalar(w1, w1, -1.0, 1.5, ALU.mult,
                                            ALU.add)
                    rn = small.tile([1, 512], F32R, tag="rns", name=f"rns{jh}")
                    nc.vector.tensor_tensor(rn[:], y1, w1, ALU.mult)
                    # broadcast across partitions via K=1 PE matmul
                    rnb = ps1.tile([128, 512], F32, tag="rnb", bufs=1,
                                   name=f"rnb{jh}")
                    nc.tensor.matmul(rnb[:], onesr[:], rn[:],
                                     start=True, stop=True)
                    # normalize (x16) to fp8 and ship this half
                    for kt in range(KT):
                        nc.vector.tensor_tensor(
                            znT[:, kt, jh * 512:(jh + 1) * 512],
                            zT[:, kt, jh * 512:(jh + 1) * 512],
                            rnb[:], ALU.mult)
                    nc.sync.dma_start(
                        ag_in[jh].rearrange("(kt p) j -> p kt j", p=128),
                        znT[:, :, jh * 512:(jh + 1) * 512])
                    if no_collective:
                        for c in range(N_CORES):
                            nc.sync.dma_start(
                                ag_out[jh][c * D:(c + 1) * D, :], ag_in[jh][:])
                    else:
                        nc.gpsimd.collective_compute(
                            "AllGather", ALU.bypass,
                            replica_groups=[list(range(N_CORES))],
                            ins=[ag_in[jh][:]], outs=[ag_out[jh][:]])

            # ---- phase 2: symmetric blockwise cos-sim (fp8 DoubleRow) ----
            rowsum = const.tile([128, MT, NSLOT], F32)
            pos_acc = const.tile([128, MT], F32)
            corr_acc = const.tile([128, MT], F32)

            pid = nc.sync.partition_id()
            pid_act = nc.scalar.partition_id()

            def gemm_tile(pg, rhs_ap, m):
                for t in range(KT // 2):
                    nc.tensor.matmul(
                        pg[:],
                        znT[:, 2 * t:2 * t + 2, m * 128:(m + 1) * 128],
                        rhs_ap[:, 2 * t:2 * t + 2, :],
                        start=(t == 0), stop=(t == KT // 2 - 1),
                        perf_mode=DR)

            with tc.tile_pool(name="rhs", bufs=2) as rhsp, \
                 tc.tile_pool(name="junk", bufs=3) as junkp, \
                 tc.tile_pool(name="dtmp", bufs=4) as dtmpp, \
                 tc.tile_pool(name="ps2", bufs=1, space="PSUM") as ps2:

                # local block d=0 first — overlaps the collectives
                for nb in range(2):
                    for m in range(MT):
                        pg = ps2.tile([128, 512], F32, tag="pg", bufs=5,
                                      name=f"pgl{nb}_{m}")
                        gemm_tile(pg, znT[:, :, nb * 512:(nb + 1) * 512], m)
                        junk = junkp.tile([128, 512], BF16, tag="junk")
                        nc.scalar.activation(
                            junk[:], pg[:], AF.Exp, scale=SCL,
                            accum_out=rowsum[:, m, nb:nb + 1])
                        if nb == m // 4:
                            # self-similarity at compile-time position
                            off = (m % 4) * 128
                            jd = junkp.tile([128, 128], F32, tag="jd")
                            nc.vector.tensor_tensor(
                                jd[:], pg[:, off:off + 128], ident[:],
                                ALU.mult)
                            d = dtmpp.tile([128, 1], F32, tag="d")
                            nc.vector.reduce_sum(d[:], jd[:],
                                                 axis=mybir.AxisListType.X)
                            nc.scalar.activation(
                                corr_acc[:, m:m + 1], d[:], AF.Exp,
                                scale=SCL)

                # remote blocks d = 1..4 per gathered chunk k; the last
                # (k=1, d=4) is emitted after the ReduceScatter so the RS
                # overlaps its compute
                def remote_block(k, dd):
                    slot = 2 + (dd - 1) * 2 + k
                    eng = nc.sync if dd % 2 == 0 else nc.scalar
                    epid = pid if dd % 2 == 0 else pid_act
                    row0 = ((epid + dd) % N_CORES) * D
                    rhs = rhsp.tile([128, KT, 512], GDT, tag="rhs")
                    src = ag_out[k][bass.ds(row0, D), :].rearrange(
                        "(kt p) j -> p kt j", p=128)
                    eng.dma_start(rhs[:], src)
                    cs = None
                    if dd < 4:
                        cs = ps2.tile([1, 512], F32, tag="cs", bufs=2,
                                      name=f"cs{k}_{dd}")
                    for m in range(MT):
                        pg = ps2.tile([128, 512], F32, tag="pg", bufs=5,
                                      name=f"pg{k}_{dd}_{m}")
                        gemm_tile(pg, rhs, m)
                        junk = junkp.tile([128, 512], BF16, tag="junk")
                        nc.scalar.activation(
                            junk[:], pg[:], AF.Exp, scale=SCL,
                            accum_out=rowsum[:, m, slot:slot + 1])
                        if dd < 4:
                            nc.tensor.matmul(cs[:], ones_b[:], junk[:],
                                             start=(m == 0),
                                             stop=(m == MT - 1))
                        if dd == 4 and k == m // 4:
                            # positive-pair logit on the block diagonal
                            off = (m % 4) * 128
                            jd = junkp.tile([128, 128], F32, tag="jd")
                            nc.vector.tensor_tensor(
                                jd[:], pg[:, off:off + 128], ident[:],
                                ALU.mult)
                            dpos = dtmpp.tile([128, 1], F32, tag="dp")
                            nc.vector.reduce_sum(
                                dpos[:], jd[:], axis=mybir.AxisListType.X)
                            nc.vector.tensor_scalar_mul(
                                pos_acc[:, m:m + 1], dpos[:], SCL)
                    if dd < 4:
                        # ship this block's column sums to core (c+dd)
                        css = dtmpp.tile([1, 512], F32, tag="css",
                                         name=f"css{k}_{dd}")
                        nc.vector.tensor_copy(css[:], cs[:])
                        nc.sync.dma_start(
                            rs_in[bass.ds((pid + dd) % N_CORES, 1),
                                  k * 512:(k + 1) * 512],
                            css[:])

                for k, dd in [(0, 1), (0, 2), (0, 3), (0, 4),
                              (1, 1), (1, 2), (1, 3)]:
                    remote_block(k, dd)

                # sum exchanged column contributions (overlaps k=1 d=4)
                if no_collective:
                    nc.sync.dma_start(rs_out[None, :], rs_in[0:1, :])
                else:
                    nc.gpsimd.collective_compute(
                        "ReduceScatter", ALU.add,
                        replica_groups=[list(range(N_CORES))],
                        ins=[rs_in[:]], outs=[rs_out[:]])

                remote_block(1, 4)

                # ---- finale: lse, nll, partial sum (batched over m) ----
                rs = small.tile([128, MT], F32)
                nc.vector.reduce_sum(rs[:], rowsum[:],
                                     axis=mybir.AxisListType.X)
                rcv = small.tile([128, MT], F32)
                nc.sync.dma_start(rcv[:], rs_out.rearrange("(m p) -> p m", p=128))
                nc.vector.tensor_tensor(rs[:], rs[:], rcv[:], ALU.add)
                nc.vector.tensor_tensor(rs[:], rs[:], corr_acc[:], ALU.subtract)
                lse = small.tile([128, MT], F32)
                nc.scalar.activation(lse[:], rs[:], AF.Ln)
                nll = small.tile([128, MT], F32)
                nc.vector.tensor_tensor(nll[:], lse[:], pos_acc[:], ALU.subtract)
                pf = ps2.tile([1, MT], F32, tag="pf", bufs=1)
                nc.tensor.matmul(pf[:], ones_f[:], nll[:], start=True, stop=True)
                fs = small.tile([1, MT], F32)
                nc.vector.tensor_copy(fs[:], pf[:])
                nc.sync.dma_start(out[:], fs[:])

    nc.compile()
    return nc


def kernel(x, w, b):
    if "nc" not in _cached:
        _cached["nc"] = _build()
    nc = _cached["nc"]
    x = np.ascontiguousarray(np.asarray(x, dtype=np.float32))
    w = np.ascontiguousarray(np.asarray(w, dtype=np.float32))
    b = np.ascontiguousarray(np.asarray(b, dtype=np.float32))
    in_maps = [{
        "xs": np.ascontiguousarray(x[c * R:(c + 1) * R]),
        "w": w, "b": b,
    } for c in range(N_CORES)]
    res = run_bass_kernel_spmd(nc, in_maps, list(range(N_CORES)))
    total = 0.0
    for c in range(N_CORES):
        total += float(res.results[c]["out"].astype(np.float64).sum())
    return np.float32(total / B)


# revision 13
# speedup vs baseline: 1.4435x; 1.1163x over previous
"""Contrastive projection head loss on 8 Trainium2 NeuronCores.

Reference computation (B=8192, E=1024, P=512):
    z_codon = relu(x[:, :E]) @ w + b          # [B, P]
    z_amino = relu(x[:, E:]) @ w + b          # [B, P]
    z  = concat([z_codon, z_amino], axis=1)   # [B, 2P]
    zn = z / max(||z||, 1e-8)
    s  = (zn @ zn.T);  s[i,i] = -9e15;  s /= 0.1
    nll_i = -s[i, (i - B/2) % B] + logsumexp(s[i, :])
    out = mean(nll)

Distribution: data-parallel over B (1024 rows/core). Each core projects and
normalizes its rows, keeping them feature-major (zn^T) as the K-major
operand of the similarity GEMM, and all-gathers zn^T in fp8 (scaled by 16)
in two column chunks pipelined with phase 1 by row-halves. Cores launch
with tens of us of skew, so the first collective completes only when the
last core arrives; tiny warmup AllGather+ReduceScatter at program start
absorb the one-time setup, and everything before the real gathers sits on
the last core's critical path — hence all 8 x-row loads are issued before
any constant setup, w is loaded through the gpsimd software DGE (casting
f32->bf16 in flight, off the HWDGE queues), and small stores ride gpsimd.

Phase 1 runs in bf16: x rows are relu'd+cast on DVE, transposed on the PE
(bf16 identity, 1 cycle/row), projected with a bf16 GEMM. 16/||z|| is
computed with a Quake-style rsqrt (bit trick + 2 Newton steps) entirely on
DVE so the Scalar engine's activation table stays on Exp the whole kernel
(table swaps cost ~2-11us), then broadcast across partitions with a K=1 PE
matmul (no DRAM round-trip).

The similarity matrix is symmetric, so each core computes only block
columns d = 0..4 (its own rows against cores c..c+4 mod 8). The GEMM runs
in fp8e4 with MatmulPerfMode.DoubleRow (two 128-K tiles per instruction).
Since operands carry a 16x scale, exp scales use 1/(T*16^2). Blocks d=1..3
also produce column sums of exp (ones-matmul per tile) routed to the
owning cores with a ReduceScatter issued BEFORE the two d=4 blocks so it
overlaps their compute; block d=4 is computed by both endpoints (row sums
only). Remote operands are addressed with partition-id-derived dynamic DMA
offsets, keeping the SPMD program identical on all cores. The
self-similarity term is removed by subtracting its exp (block d=0,
compile-time positions); the positive-pair logit is the block diagonal of
the d=4 block.

Returns per-core partial sums [1, 8]; host sums and divides by B.
"""
import numpy as np

from concourse import bass, mybir, tile, bacc
from concourse.bass_utils import run_bass_kernel_spmd
from concourse.masks import make_identity

N_CORES = 8
B = 8192
E = 1024          # embedding size (per half)
P = 512           # projection size
D = 2 * P         # z feature dim = 1024
R = B // N_CORES  # rows per core = 1024
KT = D // 128     # feature sub-tiles = 8
MT = R // 128     # row sub-tiles per core = 8
INV_T = 10.0      # 1 / temperature
S = 16.0          # fp8 operand scale: znT holds 16*zn
SCL = INV_T / (S * S)   # exp input scale: logits = SCL * (16zn . 16zn)
MAGIC = 0x5F3759DF      # Quake rsqrt seed constant

F32 = mybir.dt.float32
F32R = mybir.dt.float32r
I32 = mybir.dt.int32
BF16 = mybir.dt.bfloat16
GDT = mybir.dt.float8e4   # gathered zn^T / similarity-GEMM operand dtype
AF = mybir.ActivationFunctionType
ALU = mybir.AluOpType
DR = mybir.MatmulPerfMode.DoubleRow

NSLOT = 10        # rowsum slots: 2 local (d=0) + 8 remote (d=1..4, k=0..1)

_cached = {}


def _build(no_collective=False):
    nc = bacc.Bacc("TRN2", target_bir_lowering=False, debug=False,
                   enable_asserts=False, num_devices=N_CORES)
    x_in = nc.dram_tensor("xs", [R, 2 * E], F32, kind="ExternalInput").ap()
    w_in = nc.dram_tensor("w", [E, P], F32, kind="ExternalInput").ap()
    b_in = nc.dram_tensor("b", [P], F32, kind="ExternalInput").ap()
    out = nc.dram_tensor("out", [1, MT], F32, kind="ExternalOutput").ap()

    with tile.TileContext(nc) as tc:
        with tc.tile_pool(name="const", bufs=1) as const, \
             tc.tile_pool(name="big", bufs=1) as big, \
             tc.tile_pool(name="small", bufs=1) as small, \
             tc.tile_pool(name="dram", bufs=1, space="DRAM") as dram:

            # z^T feature-major bf16; znT is the normalized 16x-scaled fp8
            zT = big.tile([128, KT, R], BF16, tag="zT")
            znT = big.tile([128, KT, R], GDT, tag="znT")
            ag_in = [dram.tile([D, 512], GDT, name=f"ag_in{k}")
                     for k in range(2)]
            ag_out = [dram.tile([N_CORES * D, 512], GDT, name=f"ag_out{k}",
                                addr_space="Local" if no_collective else "Shared")
                      for k in range(2)]
            warm_in = dram.tile([8, 4], F32)
            warm_out = dram.tile([N_CORES * 8, 4], F32,
                                 addr_space="Local" if no_collective else "Shared")
            warm2_out = dram.tile([4], F32)
            rs_in = dram.tile([N_CORES, R], F32)
            rs_out = dram.tile([R], F32)

            with tc.tile_pool(name="xrow", bufs=8) as xrowp, \
                 tc.tile_pool(name="xbf", bufs=3) as xbfp, \
                 tc.tile_pool(name="xTp", bufs=2) as xTp, \
                 tc.tile_pool(name="sqp", bufs=2) as sqp, \
                 tc.tile_pool(name="ps1", bufs=1, space="PSUM") as ps1:

                # issue ALL x-row loads first — everything pre-gather is on
                # the last core's critical path
                xrows = []
                for rg in range(8):
                    ld = nc.sync if rg % 2 == 0 else nc.scalar
                    xrow = xrowp.tile([128, 2 * E], F32, tag="xrow",
                                      name=f"xrow{rg}")
                    ld.dma_start(xrow[:], x_in[rg * 128:(rg + 1) * 128, :])
                    xrows.append(xrow)

                # w loaded+cast f32->bf16 by the gpsimd software DGE, off
                # the HWDGE queues carrying x
                w_b = const.tile([128, E // 128, P], BF16)
                nc.gpsimd.dma_start(
                    w_b[:], w_in.rearrange("(kt p) q -> p kt q", p=128))
                b2 = const.tile([128, P // 128], F32)
                nc.gpsimd.dma_start(b2[:],
                                    b_in.rearrange("(mt p) -> p mt", p=128))

                # warmup: preload the Exp activation table while DMAs run
                dumm = const.tile([1, 8], F32)
                nc.vector.memset(dumm[:], 0.0)
                dumm2 = const.tile([1, 8], F32)
                nc.scalar.activation(dumm2[:], dumm[:], AF.Exp)

                # warmup collectives: absorb one-time CC setup latency
                wz = small.tile([8, 4], F32)
                nc.vector.memset(wz[:], 0.0)
                nc.gpsimd.dma_start(warm_in[:], wz[:])
                if not no_collective:
                    nc.gpsimd.collective_compute(
                        "AllGather", ALU.bypass,
                        replica_groups=[list(range(N_CORES))],
                        ins=[warm_in[:]], outs=[warm_out[:]])
                    nc.gpsimd.collective_compute(
                        "ReduceScatter", ALU.add,
                        replica_groups=[list(range(N_CORES))],
                        ins=[warm_in[:]], outs=[warm2_out[:]])

                # zero the ReduceScatter input (unwritten slots must be 0)
                zb = small.tile([N_CORES, R], F32)
                nc.vector.memset(zb[:], 0.0)
                nc.gpsimd.dma_start(rs_in[:], zb[:])

                ident = const.tile([128, 128], F32)
                make_identity(nc, ident[:])
                ident_b = const.tile([128, 128], BF16)
                nc.vector.tensor_copy(ident_b[:], ident[:])
                ones_f = const.tile([128, 1], F32)
                nc.vector.memset(ones_f[:], 1.0)
                ones_b = const.tile([128, 1], BF16)
                nc.vector.tensor_copy(ones_b[:], ones_f[:])
                # row-vector of ones for the PE norm broadcast (K=1 matmul)
                onesr_f = const.tile([1, 128], F32)
                nc.vector.memset(onesr_f[:], 1.0)
                onesr = const.tile([1, 128], F32R)
                nc.vector.tensor_copy(onesr[:], onesr_f[:])
                magic = const.tile([1, 512], I32)
                nc.vector.memset(magic[:], MAGIC)

                # ---- phase 1, pipelined over row-halves jh ----
                for jh in range(2):
                    # relu-cast + PE-transpose this half's rows
                    xT = xTp.tile([128, 2 * KT, 512], BF16, tag="xT",
                                  name=f"xT{jh}")
                    for r in range(4):
                        rg = jh * 4 + r
                        xbf = xbfp.tile([128, 2 * E], BF16, tag="xbf")
                        nc.vector.tensor_scalar_max(xbf[:], xrows[rg][:], 0.0)
                        for cg in range(2 * E // 512):
                            pt = ps1.tile([128, 4, 128], BF16, tag="tp",
                                          bufs=3)
                            for q in range(4):
                                ct = cg * 4 + q
                                nc.tensor.transpose(
                                    pt[:, q, :],
                                    xbf[:, ct * 128:(ct + 1) * 128],
                                    ident_b[:])
                            nc.vector.tensor_copy(
                                xT[:, cg * 4:(cg + 1) * 4,
                                   r * 128:(r + 1) * 128],
                                pt[:])
                    # project this half: zT[:, h*4+m4, jh*512:...]
                    for h in range(2):
                        for m4 in range(P // 128):
                            pz = ps1.tile([128, 512], F32, tag="pz", bufs=2)
                            for kt in range(E // 128):
                                nc.tensor.matmul(
                                    pz[:],
                                    w_b[:, kt, m4 * 128:(m4 + 1) * 128],
                                    xT[:, h * KT + kt, :],
                                    start=(kt == 0), stop=(kt == E // 128 - 1))
                            nc.vector.tensor_scalar(
                                zT[:, h * 4 + m4, jh * 512:(jh + 1) * 512],
                                pz[:], b2[:, m4:m4 + 1], None, ALU.add)
                    # row norms for this half (bf16 squares, PE reduce)
                    pn = ps1.tile([1, 512], F32, tag="pn", bufs=2,
                                  name=f"pn{jh}")
                    for kt in range(KT):
                        sq = sqp.tile([128, 512], BF16, tag="sq")
                        zsl = zT[:, kt, jh * 512:(jh + 1) * 512]
                        nc.vector.tensor_tensor(sq[:], zsl, zsl, ALU.mult)
                        nc.tensor.matmul(pn[:], ones_b[:], sq[:],
                                         start=(kt == 0), stop=(kt == KT - 1))
                    # rn = S/||z|| = rsqrt(pn/S^2): Quake seed + 2 Newton
                    # steps, all on DVE (keeps ACT's table on Exp)
                    scr = small.tile([1, 8, 512], F32, tag="scr",
                                     name=f"scr{jh}")
                    pnf, hh, t1, u1, w1, y1 = (scr[:, i, :] for i in range(6))
                    nc.vector.tensor_scalar_mul(pnf, pn[:], 1.0 / (S * S))
                    nc.vector.tensor_scalar_mul(hh, pn[:], 0.5 / (S * S))
                    nc.vector.tensor_scalar(u1.bitcast(I32), pnf.bitcast(I32),
                                            1, None, ALU.arith_shift_right)
                    nc.vector.tensor_tensor(t1.bitcast(I32), magic[:],
                                            u1.bitcast(I32), ALU.subtract)
                    # t1 now holds y0; iterate y <- y*(1.5 - h*y*y)
                    nc.vector.tensor_tensor(u1, t1, t1, ALU.mult)
                    nc.vector.tensor_tensor(w1, u1, hh, ALU.mult)
                    nc.vector.tensor_scalar(w1, w1, -1.0, 1.5, ALU.mult,
                                            ALU.add)
                    nc.vector.tensor_tensor(y1, t1, w1, ALU.mult)
                    nc.vector.tensor_tensor(u1, y1, y1, ALU.mult)
                    nc.vector.tensor_tensor(w1, u1, hh, ALU.mult)
                    nc.vector.tensor_scalar(w1, w1, -1.0, 1.5, ALU.mult,
                                            ALU.add)
                    rn = small.tile([1, 512], F32R, tag="rns", name=f"rns{jh}")
                    nc.vector.tensor_tensor(rn[:], y1, w1, ALU.mult)
                    # broadcast across partitions via K=1 PE matmul
                    rnb = ps1.tile([128, 512], F32, tag="rnb", bufs=1,
                                   name=f"rnb{jh}")
                    nc.tensor.matmul(rnb[:], onesr[:], rn[:],
                                     start=True, stop=True)
                    # normalize (x16) to fp8 and ship this half
                    for kt in range(KT):
                        nc.vector.tensor_tensor(
                            znT[:, kt, jh * 512:(jh + 1) * 512],
                            zT[:, kt, jh * 512:(jh + 1) * 512],
                            rnb[:], ALU.mult)
                    nc.sync.dma_start(
                        ag_in[jh].rearrange("(kt p) j -> p kt j", p=128),
                        znT[:, :, jh * 512:(jh + 1) * 512])
                    if no_collective:
                        for c in range(N_CORES):
                            nc.sync.dma_start(
                                ag_out[jh][c * D:(c + 1) * D, :], ag_in[jh][:])
                    else:
                        nc.gpsimd.collective_compute(
                            "AllGather", ALU.bypass,
                            replica_groups=[list(range(N_CORES))],
                            ins=[ag_in[jh][:]], outs=[ag_out[jh][:]])

            # ---- phase 2: symmetric blockwise cos-sim (fp8 DoubleRow) ----
            rowsum = const.tile([128, MT, NSLOT], F32)
            pos_acc = const.tile([128, MT], F32)
            corr_acc = const.tile([128, MT], F32)

            pid = nc.sync.partition_id()

            def gemm_tile(pg, rhs_ap, m):
                for t in range(KT // 2):
                    nc.tensor.matmul(
                        pg[:],
                        znT[:, 2 * t:2 * t + 2, m * 128:(m + 1) * 128],
                        rhs_ap[:, 2 * t:2 * t + 2, :],
                        start=(t == 0), stop=(t == KT // 2 - 1),
                        perf_mode=DR)

            with tc.tile_pool(name="rhs", bufs=2) as rhsp, \
                 tc.tile_pool(name="junk", bufs=3) as junkp, \
                 tc.tile_pool(name="dtmp", bufs=4) as dtmpp, \
                 tc.tile_pool(name="ps2", bufs=1, space="PSUM") as ps2:

                # local block d=0 first — overlaps the collectives
                for nb in range(2):
                    for m in range(MT):
                        pg = ps2.tile([128, 512], F32, tag="pg", bufs=5,
                                      name=f"pgl{nb}_{m}")
                        gemm_tile(pg, znT[:, :, nb * 512:(nb + 1) * 512], m)
                        junk = junkp.tile([128, 512], BF16, tag="junk")
                        nc.scalar.activation(
                            junk[:], pg[:], AF.Exp, scale=SCL,
                            accum_out=rowsum[:, m, nb:nb + 1])
                        if nb == m // 4:
                            # self-similarity at compile-time position
                            off = (m % 4) * 128
                            jd = junkp.tile([128, 128], F32, tag="jd")
                            nc.vector.tensor_tensor(
                                jd[:], pg[:, off:off + 128], ident[:],
                                ALU.mult)
                            d = dtmpp.tile([128, 1], F32, tag="d")
                            nc.vector.reduce_sum(d[:], jd[:],
                                                 axis=mybir.AxisListType.X)
                            nc.scalar.activation(
                                corr_acc[:, m:m + 1], d[:], AF.Exp,
                                scale=SCL)

                # remote blocks d = 1..4 per gathered chunk k; both d=4
                # blocks are emitted after the ReduceScatter so the RS
                # overlaps their compute (they produce no column sums)
                def remote_block(k, dd):
                    # NB: rhs loads wait on the AllGather — they must stay
                    # off the ACT queue or the wait blocks every queued exp
                    slot = 2 + (dd - 1) * 2 + k
                    row0 = ((pid + dd) % N_CORES) * D
                    rhs = rhsp.tile([128, KT, 512], GDT, tag="rhs")
                    src = ag_out[k][bass.ds(row0, D), :].rearrange(
                        "(kt p) j -> p kt j", p=128)
                    nc.sync.dma_start(rhs[:], src)
                    cs = None
                    if dd < 4:
                        cs = ps2.tile([1, 512], F32, tag="cs", bufs=2,
                                      name=f"cs{k}_{dd}")
                    for m in range(MT):
                        pg = ps2.tile([128, 512], F32, tag="pg", bufs=5,
                                      name=f"pg{k}_{dd}_{m}")
                        gemm_tile(pg, rhs, m)
                        junk = junkp.tile([128, 512], BF16, tag="junk")
                        nc.scalar.activation(
                            junk[:], pg[:], AF.Exp, scale=SCL,
                            accum_out=rowsum[:, m, slot:slot + 1])
                        if dd < 4:
                            nc.tensor.matmul(cs[:], ones_b[:], junk[:],
                                             start=(m == 0),
                                             stop=(m == MT - 1))
                        if dd == 4 and k == m // 4:
                            # positive-pair logit on the block diagonal
                            off = (m % 4) * 128
                            jd = junkp.tile([128, 128], F32, tag="jd")
                            nc.vector.tensor_tensor(
                                jd[:], pg[:, off:off + 128], ident[:],
                                ALU.mult)
                            dpos = dtmpp.tile([128, 1], F32, tag="dp")
                            nc.vector.reduce_sum(
                                dpos[:], jd[:], axis=mybir.AxisListType.X)
                            nc.vector.tensor_scalar_mul(
                                pos_acc[:, m:m + 1], dpos[:], SCL)
                    if dd < 4:
                        # ship this block's column sums to core (c+dd)
                        css = dtmpp.tile([1, 512], F32, tag="css",
                                         name=f"css{k}_{dd}")
                        nc.vector.tensor_copy(css[:], cs[:])
                        nc.sync.dma_start(
                            rs_in[bass.ds((pid + dd) % N_CORES, 1),
                                  k * 512:(k + 1) * 512],
                            css[:])

                for k, dd in [(0, 1), (0, 2), (0, 3),
                              (1, 1), (1, 2), (1, 3)]:
                    remote_block(k, dd)

                # sum exchanged column contributions (overlaps both d=4
                # blocks, which produce no column sums)
                if no_collective:
                    nc.sync.dma_start(rs_out[None, :], rs_in[0:1, :])
                else:
                    nc.gpsimd.collective_compute(
                        "ReduceScatter", ALU.add,
                        replica_groups=[list(range(N_CORES))],
                        ins=[rs_in[:]], outs=[rs_out[:]])

                remote_block(0, 4)
                remote_block(1, 4)

                # ---- finale: lse, nll, partial sum (batched over m) ----
                rs = small.tile([128, MT], F32)
                nc.vector.reduce_sum(rs[:], rowsum[:],
                                     axis=mybir.AxisListType.X)
                rcv = small.tile([128, MT], F32)
                nc.sync.dma_start(rcv[:], rs_out.rearrange("(m p) -> p m", p=128))
                nc.vector.tensor_tensor(rs[:], rs[:], rcv[:], ALU.add)
                nc.vector.tensor_tensor(rs[:], rs[:], corr_acc[:], ALU.subtract)
                lse = small.tile([128, MT], F32)
                nc.scalar.activation(lse[:], rs[:], AF.Ln)
                nll = small.tile([128, MT], F32)
                nc.vector.tensor_tensor(nll[:], lse[:], pos_acc[:], ALU.subtract)
                pf = ps2.tile([1, MT], F32, tag="pf", bufs=1)
                nc.tensor.matmul(pf[:], ones_f[:], nll[:], start=True, stop=True)
                fs = small.tile([1, MT], F32)
                nc.vector.tensor_copy(fs[:], pf[:])
                nc.sync.dma_start(out[:], fs[:])

    nc.compile()
    return nc


def kernel(x, w, b):
    if "nc" not in _cached:
        _cached["nc"] = _build()
    nc = _cached["nc"]
    x = np.ascontiguousarray(np.asarray(x, dtype=np.float32))
    w = np.ascontiguousarray(np.asarray(w, dtype=np.float32))
    b = np.ascontiguousarray(np.asarray(b, dtype=np.float32))
    in_maps = [{
        "xs": np.ascontiguousarray(x[c * R:(c + 1) * R]),
        "w": w, "b": b,
    } for c in range(N_CORES)]
    res = run_bass_kernel_spmd(nc, in_maps, list(range(N_CORES)))
    total = 0.0
    for c in range(N_CORES):
        total += float(res.results[c]["out"].astype(np.float64).sum())
    return np.float32(total / B)
